# revision 10
# baseline (speedup 1.0000x reference)
"""Trainium2 Bass kernel for DenseGraphSimpleOpEdgeFlow (GNN message passing).

Reference semantics (per batch b):
  support = inputs @ weight                                    [N, F]
  op_emb[diag] = self_op_emb; adjP = adj + I
  attn = sigmoid(op_emb @ attn_w + attn_b)                     [N, N, F]
  attn = (adjP != 0) * attn;  attn = 1 where adjP == 1 (skip)
  out[i, :] = sum_j attn[i, j, :] * support[j, :] + support[i, :]

Sharding: data-parallel over batch B=64 across 8 cores (8 batches/core).

Per-core device mapping:
  * z = W_aug^T @ op_emb^T on TensorE in fp32r (1 cyc/col). The moving tensor
    carries one augmented contraction row with values in {0, -100}: -100 for
    edges with adjP in {0, 1}, which drives sigmoid(z - 100) to exactly 0,
    implementing both the nonzero-mask and the removal of skip edges from the
    sigmoid path. Paired with an all-ones row in the stationary.
  * sigmoid on ScalarE straight out of PSUM, attn_b as per-partition bias,
    bf16 output.
  * skip/identity term S1[d,(b,i)] = sum_j support[j,d] * (skipmask+I)[j,i]
    as a small TensorE matmul per batch (skip mask built on device from adj).
  * contraction sum_j sigma[d,(b,i,j)] * support_T[d,(b,j)] on DVE in bf16
    2x mode: one broadcast multiply + a pairwise j-tree + tensor_reduce,
    then + S1 into the output.
  * TensorE transpose of [d, (b,i)] -> [(b,i), d] for contiguous output DMA.

The moving tensor is DMA'd as two 49-partition halves at partition bases 0
and 64 (tile_position row packing), which spreads one batch's load over all
16 SDMA engines and feeds two concurrent K=49 matmul row-tiles.
"""

import numpy as np

B, N, IN_F, OUT_F, OP_D = 64, 96, 128, 128, 48
NCORES = 8
BPC = B // NCORES  # batches per core
HALF = (N // 2) * N  # 4608 columns per half
NEG = -100.0

_CACHE = {}


def _build_nc():
    import concourse.bass as bass
    import concourse.bacc as bacc
    import concourse.tile as tile
    from concourse import mybir
    from contextlib import ExitStack

    f32 = mybir.dt.float32
    f32r = mybir.dt.float32r
    bf16 = mybir.dt.bfloat16
    i32 = mybir.dt.int32
    MUL = mybir.AluOpType.mult
    ADD = mybir.AluOpType.add
    ISEQ = mybir.AluOpType.is_equal

    nc = bacc.Bacc(None, target_bir_lowering=False)

    # ---- DRAM parameters (per-core shard views, host-marshaled) ----
    # op4[b, h] is [49, HALF]: rows 0-47 = op_emb^T (with diagonal replaced on
    # device), row 48 = mask logit row m2 in {0, -100}.
    p_op4 = nc.declare_dram_parameter("op4", [BPC, 2, OP_D + 1, HALF], f32,
                                      isOutput=False)
    p_adjs = nc.declare_dram_parameter("adjs", [N, BPC, N], i32, isOutput=False)  # [j, b, i]
    p_inpt = nc.declare_dram_parameter("inpt", [IN_F, BPC * N], f32, isOutput=False)  # inputs^T
    p_w2 = nc.declare_dram_parameter("w2", [128, 128], f32, isOutput=False)  # aug attn_w x2
    p_wgt = nc.declare_dram_parameter("wgt", [IN_F, OUT_F], f32, isOutput=False)
    p_ident = nc.declare_dram_parameter("ident", [128, 128], f32, isOutput=False)
    p_thr = nc.declare_dram_parameter("thr", [N, N], f32, isOutput=False)  # 1 - eye
    p_eye = nc.declare_dram_parameter("eye96", [N, N], f32, isOutput=False)
    p_attnb = nc.declare_dram_parameter("attnb", [OUT_F, 1], f32, isOutput=False)
    p_out = nc.declare_dram_parameter("out", [BPC * N, OUT_F], f32, isOutput=True)

    NB = BPC * N  # 768

    def bcast_b(ap):
        # [96, 96] const -> [96(part), BPC(b, stride 0), 96] broadcast AP
        return bass.AP(tensor=ap.tensor, offset=ap.offset,
                       ap=[ap.ap[0], [0, BPC], ap.ap[1]])

    def sub_ap(ap, extra_off, dims):
        return bass.AP(tensor=ap.tensor, offset=ap.offset + extra_off,
                       ap=[ap.ap[0]] + dims)

    with tile.TileContext(nc) as tc, ExitStack() as ctx:
        const = ctx.enter_context(tc.tile_pool(name="const", bufs=1))
        rhs_pool = ctx.enter_context(tc.tile_pool(name="rhs", bufs=4))

        SIG = mybir.ActivationFunctionType.Sigmoid

        # sigmoid ACT-table warm: first ACT instruction is a sigmoid, so the
        # one table load happens during the DMA ramp, not mid-pipeline
        attnb_sb = const.tile([OUT_F, 1], f32)
        nc.gpsimd.dma_start(out=attnb_sb[:], in_=p_attnb[:, :])
        warm_sb = const.tile([OUT_F, 1], bf16)
        nc.scalar.activation(out=warm_sb[:], in_=attnb_sb[:], func=SIG)

        # big moving-tensor loads first (cast f32 -> bf16 in the DMA)
        rts = [None] * BPC

        def load_rt(b):
            rt = rhs_pool.tile([128, HALF], bf16)
            nc.gpsimd.dma_start(out=rt[0:OP_D + 1, :], in_=p_op4[b, 0])
            nc.gpsimd.dma_start(out=rt[64:64 + OP_D + 1, :], in_=p_op4[b, 1])
            rts[b] = rt

        load_rt(0)
        load_rt(1)

        w2_sb = const.tile([128, 128], bf16)
        nc.gpsimd.dma_start(out=w2_sb[:], in_=p_w2[:, :])
        wgt_sb = const.tile([IN_F, OUT_F], bf16)
        nc.gpsimd.dma_start(out=wgt_sb[:], in_=p_wgt[:, :])
        inpt_sb = const.tile([IN_F, NB], bf16)
        nc.gpsimd.dma_start(out=inpt_sb[:], in_=p_inpt[:, :])
        thr_sb = const.tile([N, N], f32)
        nc.gpsimd.dma_start(out=thr_sb[:], in_=p_thr[:, :])
        eye_sb = const.tile([N, N], f32)
        nc.gpsimd.dma_start(out=eye_sb[:], in_=p_eye[:, :])
        ident_sb = const.tile([128, 128], f32)
        nc.gpsimd.dma_start(out=ident_sb[:], in_=p_ident[:, :])

        load_rt(2)
        load_rt(3)

        stbf_sb = const.tile([OUT_F, NB], bf16)   # support^T in bf16
        s1_sb = const.tile([OUT_F, NB], f32)      # skip+identity term
        snat_sb = const.tile([N, BPC, OUT_F], bf16)  # support natural [j, b, d]

        # ---------------- pre-phase: support, skip mask, skip term ----------------
        with tc.tile_pool(name="pre_ps", bufs=2, space="PSUM") as pps, \
             tc.tile_pool(name="pre_sb", bufs=2) as psb:
            # support^T [d, (b,j)] = weight^T @ inputs^T
            stp = pps.tile([OUT_F, NB], f32, tag="stp")
            nc.tensor.matmul(stp[:, 0:512], lhsT=wgt_sb[:],
                             rhs=inpt_sb[:, 0:512], start=True, stop=True)
            nc.tensor.matmul(stp[:, 512:NB], lhsT=wgt_sb[:],
                             rhs=inpt_sb[:, 512:NB], start=True, stop=True)
            nc.vector.tensor_copy(out=stbf_sb[:], in_=stp[:])

            # support natural [j, d] per b (stationary for the skip matmul)
            for b in range(BPC):
                pn = pps.tile([N, OUT_F], f32, tag="pn")
                nc.tensor.matmul(pn[:], lhsT=inpt_sb[:, b * N:(b + 1) * N],
                                 rhs=wgt_sb[:], start=True, stop=True)
                nc.scalar.copy(out=snat_sb[:, b, :], in_=pn[:])

            # skip mask + identity   (layout [j, b, i])
            adjs_sb = psb.tile([N, BPC, N], i32, tag="adjs")
            nc.gpsimd.dma_start(out=adjs_sb[:], in_=p_adjs[:, :, :])
            skf = psb.tile([N, BPC, N], f32, tag="skf")
            nc.vector.tensor_copy(out=skf[:], in_=adjs_sb[:])
            sk1 = psb.tile([N, BPC, N], f32, tag="sk1")
            nc.vector.tensor_tensor(out=sk1[:], in0=skf[:], in1=bcast_b(thr_sb[:]), op=ISEQ)
            skim = psb.tile([N, BPC, N], bf16, tag="skim")
            nc.vector.tensor_tensor(out=skim[:], in0=sk1[:], in1=bcast_b(eye_sb[:]), op=ADD)

            # S1[d, (b,i)] = sum_j support[j, d] * (skip+I)[j, i]
            for b in range(BPC):
                ps1 = pps.tile([OUT_F, N], f32, tag="ps1")
                nc.tensor.matmul(ps1[:], lhsT=snat_sb[:, b, :],
                                 rhs=skim[:, b, :], start=True, stop=True)
                nc.scalar.copy(out=s1_sb[:, b * N:(b + 1) * N], in_=ps1[:])

        # ---------------- main loop over batches ----------------
        sig_pool = ctx.enter_context(tc.tile_pool(name="sig", bufs=3))
        prod_pool = ctx.enter_context(tc.tile_pool(name="prod", bufs=2))
        tree_pool = ctx.enter_context(tc.tile_pool(name="tree", bufs=2))
        red_pool = ctx.enter_context(tc.tile_pool(name="red", bufs=2))
        pz = ctx.enter_context(tc.tile_pool(name="pz", bufs=2, space="PSUM"))
        ptr = ctx.enter_context(tc.tile_pool(name="ptr", bufs=2, space="PSUM"))
        outp = ctx.enter_context(tc.tile_pool(name="outp", bufs=2))
        ofin = ctx.enter_context(tc.tile_pool(name="ofin", bufs=1))

        out_fin = ofin.tile([OUT_F, NB], f32)

        NTOT = 2 * HALF  # 9216 columns per batch

        for b in range(BPC):
            if b + 4 < BPC:
                load_rt(b + 4)
            rt = rts[b]

            sig_t = sig_pool.tile([OUT_F, NTOT], bf16)
            for k in range(3):
                for h in range(2):
                    pzt = pz.tile([OUT_F, 1536], f32)
                    pbase = 64 * h
                    for s in range(3):
                        cc = k * 1536 + s * 512
                        nc.tensor.matmul(
                            pzt[:, s * 512:(s + 1) * 512],
                            lhsT=w2_sb[pbase:pbase + OP_D + 1, :],
                            rhs=rt[pbase:pbase + OP_D + 1, cc:cc + 512],
                            start=True, stop=True)
                    dst = sig_t[:, h * HALF + k * 1536: h * HALF + (k + 1) * 1536]
                    nc.scalar.activation(out=dst, in_=pzt[:], func=SIG,
                                         bias=attnb_sb[:, 0:1], scale=1.0)

            # P[d, (i,j)] = sigma * support_T (support broadcast over i)
            prod = prod_pool.tile([OUT_F, NTOT], bf16)
            st_b = stbf_sb[:, b * N:(b + 1) * N]
            st_bcast = bass.AP(tensor=st_b.tensor, offset=st_b.offset,
                               ap=[st_b.ap[0], [0, N], st_b.ap[1]])
            nc.vector.tensor_tensor(out=prod[:], in0=sig_t[:], in1=st_bcast, op=MUL)

            # pairwise tree over j: 96 -> 48 -> 24 -> 12 -> 6 -> reduce
            lvl = prod[:]
            width = N
            for _w in (48, 24, 12, 6):
                nxt = tree_pool.tile([OUT_F, N * _w], bf16, tag=f"t{_w}")
                nc.vector.tensor_tensor(
                    out=nxt[:],
                    in0=sub_ap(lvl, 0, [[width, N], [1, _w]]),
                    in1=sub_ap(lvl, _w, [[width, N], [1, _w]]),
                    op=ADD)
                lvl = nxt[:]
                width = _w
            cred = red_pool.tile([OUT_F, N], f32)
            nc.vector.tensor_reduce(out=cred[:],
                                    in_=sub_ap(lvl, 0, [[6, N], [1, 6]]),
                                    axis=mybir.AxisListType.X, op=ADD)
            nc.vector.tensor_tensor(out=out_fin[:, b * N:(b + 1) * N],
                                    in0=cred[:], in1=s1_sb[:, b * N:(b + 1) * N],
                                    op=ADD)

        # ---------------- output transpose + store ----------------
        for c in range(6):
            pt = ptr.tile([128, 128], f32)
            nc.tensor.transpose(pt[:], out_fin[:, c * 128:(c + 1) * 128], ident_sb[:])
            ot = outp.tile([128, 128], f32)
            nc.vector.tensor_copy(out=ot[:], in_=pt[:])
            nc.gpsimd.dma_start(out=p_out[c * 128:(c + 1) * 128, :], in_=ot[:])

    nc.finalize()
    return nc


def _get_nc():
    if "nc" not in _CACHE:
        _CACHE["nc"] = _build_nc()
    return _CACHE["nc"]


def marshal_core(inputs, adj, op_emb, weight, attn_w, attn_b, self_op_emb, core):
    """Build the in_map for one core (layout/dtype marshaling + mask logits)."""
    sl = slice(core * BPC, (core + 1) * BPC)
    op_sh = np.array(op_emb[sl], np.float32)              # [BPC, N, N, OP_D]
    idx = np.arange(N)
    op_sh[:, idx, idx, :] = np.asarray(self_op_emb, np.float32)
    op_t = op_sh.transpose(0, 3, 1, 2)                    # [BPC, OP_D, N(i), N(j)]
    adj_sh = np.asarray(adj[sl]).astype(np.int32)         # [BPC, N, N]
    eye = np.eye(N, dtype=np.float32)
    # mask logit row: -100 where (adj + I) in {0, 1} else 0
    adjp = adj_sh.astype(np.float32) + eye
    m2 = np.where(adjp <= 1.0, np.float32(NEG), np.float32(0.0))  # [BPC, N, N]
    op4 = np.empty((BPC, 2, OP_D + 1, HALF), np.float32)
    op4[:, :, :OP_D, :] = op_t.reshape(BPC, OP_D, 2, HALF).transpose(0, 2, 1, 3)
    op4[:, :, OP_D, :] = m2.reshape(BPC, 2, HALF)
    adjs = np.ascontiguousarray(adj_sh.transpose(2, 0, 1))  # [j, b, i]
    inpt = np.ascontiguousarray(
        np.asarray(inputs[sl], np.float32).reshape(BPC * N, IN_F).T)

    w2 = np.zeros((128, 128), np.float32)
    w2[0:OP_D] = attn_w
    w2[OP_D] = 1.0
    w2[64:64 + OP_D] = attn_w
    w2[64 + OP_D] = 1.0

    return {
        "op4": op4,
        "adjs": adjs,
        "inpt": inpt,
        "w2": w2,
        "wgt": np.ascontiguousarray(np.asarray(weight, np.float32)),
        "ident": np.eye(128, dtype=np.float32),
        "thr": np.ascontiguousarray(1.0 - eye),
        "eye96": np.ascontiguousarray(eye),
        "attnb": np.ascontiguousarray(np.asarray(attn_b, np.float32)[:, None]),
    }


def _ensure_ntff_hook():
    """Provide antenv.axon_hooks if the image lacks it (NTFF timing under axon)."""
    import sys as _sys

    try:
        from antenv.axon_hooks import get_axon_ntff_profile_hook  # noqa: F401
        return
    except ImportError:
        pass

    import contextlib
    import ctypes
    import types

    so_path = "/opt/axon/libaxon_pjrt.so"
    try:
        lib = ctypes.CDLL(so_path)
    except OSError:
        lib = None
    if lib is None or not hasattr(lib, "axon_start_nrt_profile"):
        hook = None
    else:
        lib.axon_start_nrt_profile.argtypes = [
            ctypes.POINTER(ctypes.c_int64), ctypes.c_size_t]
        lib.axon_start_nrt_profile.restype = ctypes.c_int64
        lib.axon_stop_nrt_profile.argtypes = [ctypes.c_char_p]
        lib.axon_stop_nrt_profile.restype = ctypes.c_int64

        @contextlib.contextmanager
        def hook(output_dir, device_ids):
            import jax
            jax.devices()
            if device_ids:
                ids = (ctypes.c_int64 * len(device_ids))(*device_ids)
                rc = lib.axon_start_nrt_profile(ids, len(device_ids))
            else:
                rc = lib.axon_start_nrt_profile(None, 0)
            if rc != 0:
                raise RuntimeError(f"axon_start_nrt_profile rc={rc}")
            try:
                yield
            finally:
                n = lib.axon_stop_nrt_profile(str(output_dir).encode())
                print(f"ntff profile: {n} file(s) written to {output_dir}")

    mod = types.ModuleType("antenv.axon_hooks")
    _state = {"hook": hook}
    mod.get_axon_ntff_profile_hook = lambda: _state["hook"]

    def _set(h):
        _state["hook"] = h

    mod.set_axon_ntff_profile_hook = _set
    _sys.modules["antenv.axon_hooks"] = mod


def run(inputs, adj, op_emb, weight, attn_w, attn_b, self_op_emb, trace=False):
    if trace:
        _ensure_ntff_hook()
    from concourse.bass_utils import run_bass_kernel_spmd

    nc = _get_nc()
    in_maps = [
        marshal_core(inputs, adj, op_emb, weight, attn_w, attn_b, self_op_emb, c)
        for c in range(NCORES)
    ]
    res = run_bass_kernel_spmd(nc, in_maps, core_ids=list(range(NCORES)), trace=trace)
    out = np.concatenate(
        [res.results[c]["out"].reshape(BPC, N, OUT_F) for c in range(NCORES)], axis=0)
    return np.ascontiguousarray(out, np.float32), res


def kernel(inputs, adj, op_emb, weight, attn_w, attn_b, self_op_emb):
    out, _ = run(inputs, adj, op_emb, weight, attn_w, attn_b, self_op_emb, trace=False)
    return out


# revision 12
# speedup vs baseline: 1.1758x; 1.1758x over previous
"""Trainium2 Bass kernel for DenseGraphSimpleOpEdgeFlow (GNN message passing).

Reference semantics (per batch b):
  support = inputs @ weight                                    [N, F]
  op_emb[diag] = self_op_emb; adjP = adj + I
  attn = sigmoid(op_emb @ attn_w + attn_b)                     [N, N, F]
  attn = (adjP != 0) * attn;  attn = 1 where adjP == 1 (skip)
  out[i, :] = sum_j attn[i, j, :] * support[j, :] + support[i, :]

Sharding: data-parallel over batch B=64 across 8 cores (8 batches/core).

Per-core device mapping:
  * z = W_aug^T @ op_emb^T on TensorE in fp32r (1 cyc/col). The moving tensor
    carries one augmented contraction row with values in {0, -100}: -100 for
    edges with adjP in {0, 1}, which drives sigmoid(z - 100) to exactly 0,
    implementing both the nonzero-mask and the removal of skip edges from the
    sigmoid path. Paired with an all-ones row in the stationary.
  * sigmoid on ScalarE straight out of PSUM, attn_b as per-partition bias,
    bf16 output.
  * skip/identity term S1[d,(b,i)] = sum_j support[j,d] * (skipmask+I)[j,i]
    as a small TensorE matmul per batch (skip mask built on device from adj).
  * contraction sum_j sigma[d,(b,i,j)] * support_T[d,(b,j)] on DVE in bf16
    2x mode: one broadcast multiply + a pairwise j-tree + tensor_reduce,
    then + S1 into the output.
  * TensorE transpose of [d, (b,i)] -> [(b,i), d] for contiguous output DMA.

The moving tensor is DMA'd as two 49-partition halves at partition bases 0
and 64 (tile_position row packing), which spreads one batch's load over all
16 SDMA engines and feeds two concurrent K=49 matmul row-tiles.
"""

import numpy as np

B, N, IN_F, OUT_F, OP_D = 64, 96, 128, 128, 48
NCORES = 8
BPC = B // NCORES  # batches per core
HALF = (N // 2) * N  # 4608 columns per half
NEG = -100.0

_CACHE = {}


def _build_nc():
    import concourse.bass as bass
    import concourse.bacc as bacc
    import concourse.tile as tile
    from concourse import mybir
    from contextlib import ExitStack

    f32 = mybir.dt.float32
    f32r = mybir.dt.float32r
    bf16 = mybir.dt.bfloat16
    i32 = mybir.dt.int32
    MUL = mybir.AluOpType.mult
    ADD = mybir.AluOpType.add
    ISEQ = mybir.AluOpType.is_equal

    nc = bacc.Bacc(None, target_bir_lowering=False)

    # ---- DRAM parameters (per-core shard views, host-marshaled) ----
    # op4[b, h] is [49, HALF]: rows 0-47 = op_emb^T (with diagonal replaced on
    # device), row 48 = mask logit row m2 in {0, -100}.
    p_op4 = nc.declare_dram_parameter("op4", [BPC, 2, OP_D + 1, HALF], f32,
                                      isOutput=False)
    p_adjs = nc.declare_dram_parameter("adjs", [N, BPC, N], i32, isOutput=False)  # [j, b, i]
    p_inpt = nc.declare_dram_parameter("inpt", [IN_F, BPC * N], f32, isOutput=False)  # inputs^T
    p_w2 = nc.declare_dram_parameter("w2", [128, 128], f32, isOutput=False)  # aug attn_w x2
    p_wgt = nc.declare_dram_parameter("wgt", [IN_F, OUT_F], f32, isOutput=False)
    p_ident = nc.declare_dram_parameter("ident", [128, 128], f32, isOutput=False)
    p_thr = nc.declare_dram_parameter("thr", [N, N], f32, isOutput=False)  # 1 - eye
    p_eye = nc.declare_dram_parameter("eye96", [N, N], f32, isOutput=False)
    p_attnb = nc.declare_dram_parameter("attnb", [OUT_F, 1], f32, isOutput=False)
    p_out = nc.declare_dram_parameter("out", [BPC * N, OUT_F], f32, isOutput=True)

    NB = BPC * N  # 768

    def bcast_b(ap):
        # [96, 96] const -> [96(part), BPC(b, stride 0), 96] broadcast AP
        return bass.AP(tensor=ap.tensor, offset=ap.offset,
                       ap=[ap.ap[0], [0, BPC], ap.ap[1]])

    def sub_ap(ap, extra_off, dims):
        return bass.AP(tensor=ap.tensor, offset=ap.offset + extra_off,
                       ap=[ap.ap[0]] + dims)

    with tile.TileContext(nc) as tc, ExitStack() as ctx:
        const = ctx.enter_context(tc.tile_pool(name="const", bufs=1))
        rhs_pool = ctx.enter_context(tc.tile_pool(name="rhs", bufs=4))

        SIG = mybir.ActivationFunctionType.Sigmoid

        # sigmoid ACT-table warm: first ACT instruction is a sigmoid, so the
        # one table load happens during the DMA ramp, not mid-pipeline
        attnb_sb = const.tile([OUT_F, 1], f32)
        nc.gpsimd.dma_start(out=attnb_sb[:], in_=p_attnb[:, :])
        warm_sb = const.tile([OUT_F, 1], bf16)
        nc.scalar.activation(out=warm_sb[:], in_=attnb_sb[:], func=SIG)

        # small pre-phase inputs first (~1 us total), then the big loads
        wgt_sb = const.tile([IN_F, OUT_F], bf16)
        nc.gpsimd.dma_start(out=wgt_sb[:], in_=p_wgt[:, :])
        inpt_sb = const.tile([IN_F, NB], bf16)
        nc.gpsimd.dma_start(out=inpt_sb[:], in_=p_inpt[:, :])
        adjs_sb0 = const.tile([N, BPC, N], i32)
        nc.gpsimd.dma_start(out=adjs_sb0[:], in_=p_adjs[:, :, :])
        w2_sb = const.tile([128, 128], bf16)
        nc.gpsimd.dma_start(out=w2_sb[:], in_=p_w2[:, :])
        thr_sb = const.tile([N, N], f32)
        nc.gpsimd.dma_start(out=thr_sb[:], in_=p_thr[:, :])
        eye_sb = const.tile([N, N], f32)
        nc.gpsimd.dma_start(out=eye_sb[:], in_=p_eye[:, :])

        # big moving-tensor loads (cast f32 -> bf16 in the DMA)
        rts = [None] * BPC

        def load_rt(b):
            rt = rhs_pool.tile([128, HALF], bf16)
            nc.gpsimd.dma_start(out=rt[0:OP_D + 1, :], in_=p_op4[b, 0])
            nc.gpsimd.dma_start(out=rt[64:64 + OP_D + 1, :], in_=p_op4[b, 1])
            rts[b] = rt

        load_rt(0)
        load_rt(1)
        ident_sb = const.tile([128, 128], f32)
        nc.gpsimd.dma_start(out=ident_sb[:], in_=p_ident[:, :])
        load_rt(2)
        load_rt(3)

        stbf_sb = const.tile([OUT_F, NB], bf16)   # support^T in bf16
        s1_sb = const.tile([OUT_F, NB], f32)      # skip+identity term
        snat_sb = const.tile([N, BPC, OUT_F], bf16)  # support natural [j, b, d]

        pz = ctx.enter_context(tc.tile_pool(name="pz", bufs=2, space="PSUM"))
        ptr = ctx.enter_context(tc.tile_pool(name="ptr", bufs=2, space="PSUM"))

        def zslot():
            zt = pz.tile([OUT_F, 1536], f32, tag="z")
            return zt

        # ---------------- pre-phase: support, skip mask, skip term ----------------
        with tc.tile_pool(name="pre_sb", bufs=2) as psb:
            # support^T [d, (b,j)] = weight^T @ inputs^T
            stp = zslot()
            nc.tensor.matmul(stp[:, 0:512], lhsT=wgt_sb[:],
                             rhs=inpt_sb[:, 0:512], start=True, stop=True)
            nc.tensor.matmul(stp[:, 512:NB], lhsT=wgt_sb[:],
                             rhs=inpt_sb[:, 512:NB], start=True, stop=True)
            nc.vector.tensor_copy(out=stbf_sb[:], in_=stp[:, 0:NB])

            # support natural [j, d] per b (stationary for the skip matmul)
            for b in range(BPC):
                pn = zslot()
                nc.tensor.matmul(pn[:N, 0:OUT_F], lhsT=inpt_sb[:, b * N:(b + 1) * N],
                                 rhs=wgt_sb[:], start=True, stop=True)
                nc.scalar.copy(out=snat_sb[:, b, :], in_=pn[:N, 0:OUT_F])

            # skip mask + identity   (layout [j, b, i])
            skf = psb.tile([N, BPC, N], f32, tag="skf")
            nc.vector.tensor_copy(out=skf[:], in_=adjs_sb0[:])
            sk1 = psb.tile([N, BPC, N], f32, tag="sk1")
            nc.vector.tensor_tensor(out=sk1[:], in0=skf[:], in1=bcast_b(thr_sb[:]), op=ISEQ)
            skim = psb.tile([N, BPC, N], bf16, tag="skim")
            nc.vector.tensor_tensor(out=skim[:], in0=sk1[:], in1=bcast_b(eye_sb[:]), op=ADD)

            # S1[d, (b,i)] = sum_j support[j, d] * (skip+I)[j, i]
            for b in range(BPC):
                ps1 = zslot()
                nc.tensor.matmul(ps1[:, 0:N], lhsT=snat_sb[:, b, :],
                                 rhs=skim[:, b, :], start=True, stop=True)
                nc.scalar.copy(out=s1_sb[:, b * N:(b + 1) * N], in_=ps1[:, 0:N])

        # ---------------- main loop over batches ----------------
        sig_pool = ctx.enter_context(tc.tile_pool(name="sig", bufs=3))
        prod_pool = ctx.enter_context(tc.tile_pool(name="prod", bufs=2))
        tree_pool = ctx.enter_context(tc.tile_pool(name="tree", bufs=2))
        red_pool = ctx.enter_context(tc.tile_pool(name="red", bufs=2))
        outp = ctx.enter_context(tc.tile_pool(name="outp", bufs=2))
        ofin = ctx.enter_context(tc.tile_pool(name="ofin", bufs=1))

        out_fin = ofin.tile([OUT_F, NB], f32)

        NTOT = 2 * HALF  # 9216 columns per batch

        for b in range(BPC):
            if b + 4 < BPC:
                load_rt(b + 4)
            rt = rts[b]

            sig_t = sig_pool.tile([OUT_F, NTOT], bf16)
            for k in range(3):
                for h in range(2):
                    pzt = zslot()
                    pbase = 64 * h
                    for s in range(3):
                        cc = k * 1536 + s * 512
                        nc.tensor.matmul(
                            pzt[:, s * 512:(s + 1) * 512],
                            lhsT=w2_sb[pbase:pbase + OP_D + 1, :],
                            rhs=rt[pbase:pbase + OP_D + 1, cc:cc + 512],
                            start=True, stop=True)
                    dst = sig_t[:, h * HALF + k * 1536: h * HALF + (k + 1) * 1536]
                    nc.scalar.activation(out=dst, in_=pzt[:], func=SIG,
                                         bias=attnb_sb[:, 0:1], scale=1.0)

            # P[d, (i,j)] = sigma * support_T (support broadcast over i)
            prod = prod_pool.tile([OUT_F, NTOT], bf16)
            st_b = stbf_sb[:, b * N:(b + 1) * N]
            st_bcast = bass.AP(tensor=st_b.tensor, offset=st_b.offset,
                               ap=[st_b.ap[0], [0, N], st_b.ap[1]])
            nc.vector.tensor_tensor(out=prod[:], in0=sig_t[:], in1=st_bcast, op=MUL)

            # pairwise tree over j: 96 -> 48 -> 24 -> 12 -> 6 -> reduce
            lvl = prod[:]
            width = N
            for _w in (48, 24, 12, 6):
                nxt = tree_pool.tile([OUT_F, N * _w], bf16, tag=f"t{_w}")
                nc.vector.tensor_tensor(
                    out=nxt[:],
                    in0=sub_ap(lvl, 0, [[width, N], [1, _w]]),
                    in1=sub_ap(lvl, _w, [[width, N], [1, _w]]),
                    op=ADD)
                lvl = nxt[:]
                width = _w
            cred = red_pool.tile([OUT_F, N], f32)
            nc.vector.tensor_reduce(out=cred[:],
                                    in_=sub_ap(lvl, 0, [[6, N], [1, 6]]),
                                    axis=mybir.AxisListType.X, op=ADD)
            nc.vector.tensor_tensor(out=out_fin[:, b * N:(b + 1) * N],
                                    in0=cred[:], in1=s1_sb[:, b * N:(b + 1) * N],
                                    op=ADD)

        # ---------------- output transpose + store ----------------
        for c in range(6):
            pt = ptr.tile([128, 128], f32)
            nc.tensor.transpose(pt[:], out_fin[:, c * 128:(c + 1) * 128], ident_sb[:])
            ot = outp.tile([128, 128], f32)
            nc.vector.tensor_copy(out=ot[:], in_=pt[:])
            nc.gpsimd.dma_start(out=p_out[c * 128:(c + 1) * 128, :], in_=ot[:])

    nc.finalize()
    return nc


def _get_nc():
    if "nc" not in _CACHE:
        _CACHE["nc"] = _build_nc()
    return _CACHE["nc"]


def marshal_core(inputs, adj, op_emb, weight, attn_w, attn_b, self_op_emb, core):
    """Build the in_map for one core (layout/dtype marshaling + mask logits)."""
    sl = slice(core * BPC, (core + 1) * BPC)
    op_sh = np.array(op_emb[sl], np.float32)              # [BPC, N, N, OP_D]
    idx = np.arange(N)
    op_sh[:, idx, idx, :] = np.asarray(self_op_emb, np.float32)
    op_t = op_sh.transpose(0, 3, 1, 2)                    # [BPC, OP_D, N(i), N(j)]
    adj_sh = np.asarray(adj[sl]).astype(np.int32)         # [BPC, N, N]
    eye = np.eye(N, dtype=np.float32)
    # mask logit row: -100 where (adj + I) in {0, 1} else 0
    adjp = adj_sh.astype(np.float32) + eye
    m2 = np.where(adjp <= 1.0, np.float32(NEG), np.float32(0.0))  # [BPC, N, N]
    op4 = np.empty((BPC, 2, OP_D + 1, HALF), np.float32)
    op4[:, :, :OP_D, :] = op_t.reshape(BPC, OP_D, 2, HALF).transpose(0, 2, 1, 3)
    op4[:, :, OP_D, :] = m2.reshape(BPC, 2, HALF)
    adjs = np.ascontiguousarray(adj_sh.transpose(2, 0, 1))  # [j, b, i]
    inpt = np.ascontiguousarray(
        np.asarray(inputs[sl], np.float32).reshape(BPC * N, IN_F).T)

    w2 = np.zeros((128, 128), np.float32)
    w2[0:OP_D] = attn_w
    w2[OP_D] = 1.0
    w2[64:64 + OP_D] = attn_w
    w2[64 + OP_D] = 1.0

    return {
        "op4": op4,
        "adjs": adjs,
        "inpt": inpt,
        "w2": w2,
        "wgt": np.ascontiguousarray(np.asarray(weight, np.float32)),
        "ident": np.eye(128, dtype=np.float32),
        "thr": np.ascontiguousarray(1.0 - eye),
        "eye96": np.ascontiguousarray(eye),
        "attnb": np.ascontiguousarray(np.asarray(attn_b, np.float32)[:, None]),
    }


def _ensure_ntff_hook():
    """Provide antenv.axon_hooks if the image lacks it (NTFF timing under axon)."""
    import sys as _sys

    try:
        from antenv.axon_hooks import get_axon_ntff_profile_hook  # noqa: F401
        return
    except ImportError:
        pass

    import contextlib
    import ctypes
    import types

    so_path = "/opt/axon/libaxon_pjrt.so"
    try:
        lib = ctypes.CDLL(so_path)
    except OSError:
        lib = None
    if lib is None or not hasattr(lib, "axon_start_nrt_profile"):
        hook = None
    else:
        lib.axon_start_nrt_profile.argtypes = [
            ctypes.POINTER(ctypes.c_int64), ctypes.c_size_t]
        lib.axon_start_nrt_profile.restype = ctypes.c_int64
        lib.axon_stop_nrt_profile.argtypes = [ctypes.c_char_p]
        lib.axon_stop_nrt_profile.restype = ctypes.c_int64

        @contextlib.contextmanager
        def hook(output_dir, device_ids):
            import jax
            jax.devices()
            if device_ids:
                ids = (ctypes.c_int64 * len(device_ids))(*device_ids)
                rc = lib.axon_start_nrt_profile(ids, len(device_ids))
            else:
                rc = lib.axon_start_nrt_profile(None, 0)
            if rc != 0:
                raise RuntimeError(f"axon_start_nrt_profile rc={rc}")
            try:
                yield
            finally:
                n = lib.axon_stop_nrt_profile(str(output_dir).encode())
                print(f"ntff profile: {n} file(s) written to {output_dir}")

    mod = types.ModuleType("antenv.axon_hooks")
    _state = {"hook": hook}
    mod.get_axon_ntff_profile_hook = lambda: _state["hook"]

    def _set(h):
        _state["hook"] = h

    mod.set_axon_ntff_profile_hook = _set
    _sys.modules["antenv.axon_hooks"] = mod


def run(inputs, adj, op_emb, weight, attn_w, attn_b, self_op_emb, trace=False):
    if trace:
        _ensure_ntff_hook()
    from concourse.bass_utils import run_bass_kernel_spmd

    nc = _get_nc()
    in_maps = [
        marshal_core(inputs, adj, op_emb, weight, attn_w, attn_b, self_op_emb, c)
        for c in range(NCORES)
    ]
    res = run_bass_kernel_spmd(nc, in_maps, core_ids=list(range(NCORES)), trace=trace)
    out = np.concatenate(
        [res.results[c]["out"].reshape(BPC, N, OUT_F) for c in range(NCORES)], axis=0)
    return np.ascontiguousarray(out, np.float32), res


def kernel(inputs, adj, op_emb, weight, attn_w, attn_b, self_op_emb):
    out, _ = run(inputs, adj, op_emb, weight, attn_w, attn_b, self_op_emb, trace=False)
    return out


# revision 13
# speedup vs baseline: 1.2542x; 1.0667x over previous
"""Trainium2 Bass kernel for DenseGraphSimpleOpEdgeFlow (GNN message passing).

Reference semantics (per batch b):
  support = inputs @ weight                                    [N, F]
  op_emb[diag] = self_op_emb; adjP = adj + I
  attn = sigmoid(op_emb @ attn_w + attn_b)                     [N, N, F]
  attn = (adjP != 0) * attn;  attn = 1 where adjP == 1 (skip)
  out[i, :] = sum_j attn[i, j, :] * support[j, :] + support[i, :]

Sharding: data-parallel over batch B=64 across 8 cores (8 batches/core).

Per-core device mapping:
  * z = W_aug^T @ op_emb^T on TensorE in fp32r (1 cyc/col). The moving tensor
    carries one augmented contraction row with values in {0, -100}: -100 for
    edges with adjP in {0, 1}, which drives sigmoid(z - 100) to exactly 0,
    implementing both the nonzero-mask and the removal of skip edges from the
    sigmoid path. Paired with an all-ones row in the stationary.
  * sigmoid on ScalarE straight out of PSUM, attn_b as per-partition bias,
    bf16 output.
  * skip/identity term S1[d,(b,i)] = sum_j support[j,d] * (skipmask+I)[j,i]
    as a small TensorE matmul per batch (skip mask built on device from adj).
  * contraction sum_j sigma[d,(b,i,j)] * support_T[d,(b,j)] on DVE in bf16
    2x mode: one broadcast multiply + a pairwise j-tree + tensor_reduce,
    then + S1 into the output.
  * TensorE transpose of [d, (b,i)] -> [(b,i), d] for contiguous output DMA.

The moving tensor is DMA'd as two 49-partition halves at partition bases 0
and 64 (tile_position row packing), which spreads one batch's load over all
16 SDMA engines and feeds two concurrent K=49 matmul row-tiles.
"""

import numpy as np

B, N, IN_F, OUT_F, OP_D = 64, 96, 128, 128, 48
NCORES = 8
BPC = B // NCORES  # batches per core
HALF = (N // 2) * N  # 4608 columns per half
NEG = -100.0

_CACHE = {}


def _build_nc():
    import concourse.bass as bass
    import concourse.bacc as bacc
    import concourse.tile as tile
    from concourse import mybir
    from contextlib import ExitStack

    f32 = mybir.dt.float32
    f32r = mybir.dt.float32r
    bf16 = mybir.dt.bfloat16
    i32 = mybir.dt.int32
    MUL = mybir.AluOpType.mult
    ADD = mybir.AluOpType.add
    ISEQ = mybir.AluOpType.is_equal

    nc = bacc.Bacc(None, target_bir_lowering=False)

    # ---- DRAM parameters (per-core shard views, host-marshaled) ----
    # op4[b, h] is [49, HALF]: rows 0-47 = op_emb^T (with diagonal replaced on
    # device), row 48 = mask logit row m2 in {0, -100}.
    p_op4 = nc.declare_dram_parameter("op4", [BPC, 2, OP_D + 1, HALF], f32,
                                      isOutput=False)
    p_adjs = nc.declare_dram_parameter("adjs", [N, BPC, N], i32, isOutput=False)  # [j, b, i]
    p_inpt = nc.declare_dram_parameter("inpt", [IN_F, BPC * N], f32, isOutput=False)  # inputs^T
    p_w2 = nc.declare_dram_parameter("w2", [128, 128], f32, isOutput=False)  # aug attn_w x2
    p_wgt = nc.declare_dram_parameter("wgt", [IN_F, OUT_F], f32, isOutput=False)
    p_ident = nc.declare_dram_parameter("ident", [128, 128], f32, isOutput=False)
    p_thr = nc.declare_dram_parameter("thr", [N, N], f32, isOutput=False)  # 1 - eye
    p_eye = nc.declare_dram_parameter("eye96", [N, N], f32, isOutput=False)
    p_attnb = nc.declare_dram_parameter("attnb", [OUT_F, 1], f32, isOutput=False)
    p_out = nc.declare_dram_parameter("out", [BPC * N, OUT_F], f32, isOutput=True)

    NB = BPC * N  # 768

    def bcast_b(ap):
        # [96, 96] const -> [96(part), BPC(b, stride 0), 96] broadcast AP
        return bass.AP(tensor=ap.tensor, offset=ap.offset,
                       ap=[ap.ap[0], [0, BPC], ap.ap[1]])

    def sub_ap(ap, extra_off, dims):
        return bass.AP(tensor=ap.tensor, offset=ap.offset + extra_off,
                       ap=[ap.ap[0]] + dims)

    with tile.TileContext(nc) as tc, ExitStack() as ctx:
        const = ctx.enter_context(tc.tile_pool(name="const", bufs=1))
        rhs_pool = ctx.enter_context(tc.tile_pool(name="rhs", bufs=4))

        SIG = mybir.ActivationFunctionType.Sigmoid

        # sigmoid ACT-table warm: first ACT instruction is a sigmoid, so the
        # one table load happens during the DMA ramp, not mid-pipeline
        attnb_sb = const.tile([OUT_F, 1], f32)
        nc.gpsimd.dma_start(out=attnb_sb[:], in_=p_attnb[:, :])
        warm_sb = const.tile([OUT_F, 1], bf16)
        nc.scalar.activation(out=warm_sb[:], in_=attnb_sb[:], func=SIG)

        # small pre-phase inputs first (~1 us total), then the big loads
        wgt_sb = const.tile([IN_F, OUT_F], bf16)
        nc.gpsimd.dma_start(out=wgt_sb[:], in_=p_wgt[:, :])
        inpt_sb = const.tile([IN_F, NB], bf16)
        nc.gpsimd.dma_start(out=inpt_sb[:], in_=p_inpt[:, :])
        adjs_sb0 = const.tile([N, BPC, N], i32)
        nc.gpsimd.dma_start(out=adjs_sb0[:], in_=p_adjs[:, :, :])
        w2_sb = const.tile([128, 128], bf16)
        nc.gpsimd.dma_start(out=w2_sb[:], in_=p_w2[:, :])
        thr_sb = const.tile([N, N], f32)
        nc.gpsimd.dma_start(out=thr_sb[:], in_=p_thr[:, :])
        eye_sb = const.tile([N, N], f32)
        nc.gpsimd.dma_start(out=eye_sb[:], in_=p_eye[:, :])

        # big moving-tensor loads (cast f32 -> bf16 in the DMA)
        rts = [None] * BPC

        def load_rt(b):
            rt = rhs_pool.tile([128, HALF], bf16)
            nc.gpsimd.dma_start(out=rt[0:OP_D + 1, :], in_=p_op4[b, 0])
            nc.gpsimd.dma_start(out=rt[64:64 + OP_D + 1, :], in_=p_op4[b, 1])
            rts[b] = rt

        load_rt(0)
        load_rt(1)
        ident_sb = const.tile([128, 128], f32)
        nc.gpsimd.dma_start(out=ident_sb[:], in_=p_ident[:, :])
        load_rt(2)
        load_rt(3)

        stbf_sb = const.tile([OUT_F, NB], bf16)   # support^T in bf16
        s1_sb = const.tile([OUT_F, NB], f32)      # skip+identity term
        snat_sb = const.tile([N, BPC, OUT_F], bf16)  # support natural [j, b, d]

        pz = ctx.enter_context(tc.tile_pool(name="pz", bufs=2, space="PSUM"))
        ptr = ctx.enter_context(tc.tile_pool(name="ptr", bufs=2, space="PSUM"))

        def zslot():
            zt = pz.tile([OUT_F, 1536], f32, tag="z")
            return zt

        # ---------------- pre-phase: support, skip mask, skip term ----------------
        with tc.tile_pool(name="pre_sb", bufs=2) as psb:
            # support^T [d, (b,j)] = weight^T @ inputs^T
            stp = zslot()
            nc.tensor.matmul(stp[:, 0:512], lhsT=wgt_sb[:],
                             rhs=inpt_sb[:, 0:512], start=True, stop=True)
            nc.tensor.matmul(stp[:, 512:NB], lhsT=wgt_sb[:],
                             rhs=inpt_sb[:, 512:NB], start=True, stop=True)
            nc.vector.tensor_copy(out=stbf_sb[:], in_=stp[:, 0:NB])

            # support natural [j, d] per b (stationary for the skip matmul)
            for b in range(BPC):
                pn = zslot()
                nc.tensor.matmul(pn[:N, 0:OUT_F], lhsT=inpt_sb[:, b * N:(b + 1) * N],
                                 rhs=wgt_sb[:], start=True, stop=True)
                nc.scalar.copy(out=snat_sb[:, b, :], in_=pn[:N, 0:OUT_F])

            # skip mask + identity   (layout [j, b, i])
            skf = psb.tile([N, BPC, N], f32, tag="skf")
            nc.vector.tensor_copy(out=skf[:], in_=adjs_sb0[:])
            sk1 = psb.tile([N, BPC, N], f32, tag="sk1")
            nc.vector.tensor_tensor(out=sk1[:], in0=skf[:], in1=bcast_b(thr_sb[:]), op=ISEQ)
            skim = psb.tile([N, BPC, N], bf16, tag="skim")
            nc.vector.tensor_tensor(out=skim[:], in0=sk1[:], in1=bcast_b(eye_sb[:]), op=ADD)

            # S1[d, (b,i)] = sum_j support[j, d] * (skip+I)[j, i]
            for b in range(BPC):
                ps1 = zslot()
                nc.tensor.matmul(ps1[:, 0:N], lhsT=snat_sb[:, b, :],
                                 rhs=skim[:, b, :], start=True, stop=True)
                nc.scalar.copy(out=s1_sb[:, b * N:(b + 1) * N], in_=ps1[:, 0:N])

        # ---------------- main loop over batches ----------------
        sig_pool = ctx.enter_context(tc.tile_pool(name="sig", bufs=3))
        prod_pool = ctx.enter_context(tc.tile_pool(name="prod", bufs=2))
        tree_pool = ctx.enter_context(tc.tile_pool(name="tree", bufs=2))
        red_pool = ctx.enter_context(tc.tile_pool(name="red", bufs=2))
        outp = ctx.enter_context(tc.tile_pool(name="outp", bufs=2))
        ofin = ctx.enter_context(tc.tile_pool(name="ofin", bufs=1))

        out_fin = ofin.tile([OUT_F, NB], f32)

        NTOT = 2 * HALF  # 9216 columns per batch

        for b in range(BPC):
            if b + 4 < BPC:
                load_rt(b + 4)
            rt = rts[b]

            sig_t = sig_pool.tile([OUT_F, NTOT], bf16)
            prod = prod_pool.tile([OUT_F, NTOT], bf16)
            l1t = tree_pool.tile([OUT_F, N * 48], bf16, tag="t48")
            st_b = stbf_sb[:, b * N:(b + 1) * N]
            st_bcast_h = bass.AP(tensor=st_b.tensor, offset=st_b.offset,
                                 ap=[st_b.ap[0], [0, N // 2], st_b.ap[1]])
            for h in range(2):
                pbase = 64 * h
                for k in range(3):
                    pzt = zslot()
                    for s in range(3):
                        cc = k * 1536 + s * 512
                        nc.tensor.matmul(
                            pzt[:, s * 512:(s + 1) * 512],
                            lhsT=w2_sb[pbase:pbase + OP_D + 1, :],
                            rhs=rt[pbase:pbase + OP_D + 1, cc:cc + 512],
                            start=True, stop=True)
                    dst = sig_t[:, h * HALF + k * 1536: h * HALF + (k + 1) * 1536]
                    nc.scalar.activation(out=dst, in_=pzt[:], func=SIG,
                                         bias=attnb_sb[:, 0:1], scale=1.0)
                # P = sigma * support_T for this half (48 i-blocks)
                nc.vector.tensor_tensor(
                    out=prod[:, h * HALF:(h + 1) * HALF],
                    in0=sig_t[:, h * HALF:(h + 1) * HALF],
                    in1=st_bcast_h, op=MUL)
                # first tree level for this half
                nc.vector.tensor_tensor(
                    out=l1t[:, h * HALF // 2:(h + 1) * HALF // 2],
                    in0=sub_ap(prod[:], h * HALF, [[N, N // 2], [1, 48]]),
                    in1=sub_ap(prod[:], h * HALF + 48, [[N, N // 2], [1, 48]]),
                    op=ADD)

            # remaining tree levels: 48 -> 24 -> 12 -> 6 -> reduce
            lvl = l1t[:]
            width = 48
            for _w in (24, 12, 6):
                nxt = tree_pool.tile([OUT_F, N * _w], bf16, tag=f"t{_w}")
                nc.vector.tensor_tensor(
                    out=nxt[:],
                    in0=sub_ap(lvl, 0, [[width, N], [1, _w]]),
                    in1=sub_ap(lvl, _w, [[width, N], [1, _w]]),
                    op=ADD)
                lvl = nxt[:]
                width = _w
            cred = red_pool.tile([OUT_F, N], f32)
            nc.vector.tensor_reduce(out=cred[:],
                                    in_=sub_ap(lvl, 0, [[6, N], [1, 6]]),
                                    axis=mybir.AxisListType.X, op=ADD)
            nc.vector.tensor_tensor(out=out_fin[:, b * N:(b + 1) * N],
                                    in0=cred[:], in1=s1_sb[:, b * N:(b + 1) * N],
                                    op=ADD)

        # ---------------- output transpose + store ----------------
        for c in range(6):
            pt = ptr.tile([128, 128], f32)
            nc.tensor.transpose(pt[:], out_fin[:, c * 128:(c + 1) * 128], ident_sb[:])
            ot = outp.tile([128, 128], f32)
            nc.scalar.copy(out=ot[:], in_=pt[:])
            nc.gpsimd.dma_start(out=p_out[c * 128:(c + 1) * 128, :], in_=ot[:])

    nc.finalize()
    return nc


def _get_nc():
    if "nc" not in _CACHE:
        _CACHE["nc"] = _build_nc()
    return _CACHE["nc"]


def marshal_core(inputs, adj, op_emb, weight, attn_w, attn_b, self_op_emb, core):
    """Build the in_map for one core (layout/dtype marshaling + mask logits)."""
    sl = slice(core * BPC, (core + 1) * BPC)
    op_sh = np.array(op_emb[sl], np.float32)              # [BPC, N, N, OP_D]
    idx = np.arange(N)
    op_sh[:, idx, idx, :] = np.asarray(self_op_emb, np.float32)
    op_t = op_sh.transpose(0, 3, 1, 2)                    # [BPC, OP_D, N(i), N(j)]
    adj_sh = np.asarray(adj[sl]).astype(np.int32)         # [BPC, N, N]
    eye = np.eye(N, dtype=np.float32)
    # mask logit row: -100 where (adj + I) in {0, 1} else 0
    adjp = adj_sh.astype(np.float32) + eye
    m2 = np.where(adjp <= 1.0, np.float32(NEG), np.float32(0.0))  # [BPC, N, N]
    op4 = np.empty((BPC, 2, OP_D + 1, HALF), np.float32)
    op4[:, :, :OP_D, :] = op_t.reshape(BPC, OP_D, 2, HALF).transpose(0, 2, 1, 3)
    op4[:, :, OP_D, :] = m2.reshape(BPC, 2, HALF)
    adjs = np.ascontiguousarray(adj_sh.transpose(2, 0, 1))  # [j, b, i]
    inpt = np.ascontiguousarray(
        np.asarray(inputs[sl], np.float32).reshape(BPC * N, IN_F).T)

    w2 = np.zeros((128, 128), np.float32)
    w2[0:OP_D] = attn_w
    w2[OP_D] = 1.0
    w2[64:64 + OP_D] = attn_w
    w2[64 + OP_D] = 1.0

    return {
        "op4": op4,
        "adjs": adjs,
        "inpt": inpt,
        "w2": w2,
        "wgt": np.ascontiguousarray(np.asarray(weight, np.float32)),
        "ident": np.eye(128, dtype=np.float32),
        "thr": np.ascontiguousarray(1.0 - eye),
        "eye96": np.ascontiguousarray(eye),
        "attnb": np.ascontiguousarray(np.asarray(attn_b, np.float32)[:, None]),
    }


def _ensure_ntff_hook():
    """Provide antenv.axon_hooks if the image lacks it (NTFF timing under axon)."""
    import sys as _sys

    try:
        from antenv.axon_hooks import get_axon_ntff_profile_hook  # noqa: F401
        return
    except ImportError:
        pass

    import contextlib
    import ctypes
    import types

    so_path = "/opt/axon/libaxon_pjrt.so"
    try:
        lib = ctypes.CDLL(so_path)
    except OSError:
        lib = None
    if lib is None or not hasattr(lib, "axon_start_nrt_profile"):
        hook = None
    else:
        lib.axon_start_nrt_profile.argtypes = [
            ctypes.POINTER(ctypes.c_int64), ctypes.c_size_t]
        lib.axon_start_nrt_profile.restype = ctypes.c_int64
        lib.axon_stop_nrt_profile.argtypes = [ctypes.c_char_p]
        lib.axon_stop_nrt_profile.restype = ctypes.c_int64

        @contextlib.contextmanager
        def hook(output_dir, device_ids):
            import jax
            jax.devices()
            if device_ids:
                ids = (ctypes.c_int64 * len(device_ids))(*device_ids)
                rc = lib.axon_start_nrt_profile(ids, len(device_ids))
            else:
                rc = lib.axon_start_nrt_profile(None, 0)
            if rc != 0:
                raise RuntimeError(f"axon_start_nrt_profile rc={rc}")
            try:
                yield
            finally:
                n = lib.axon_stop_nrt_profile(str(output_dir).encode())
                print(f"ntff profile: {n} file(s) written to {output_dir}")

    mod = types.ModuleType("antenv.axon_hooks")
    _state = {"hook": hook}
    mod.get_axon_ntff_profile_hook = lambda: _state["hook"]

    def _set(h):
        _state["hook"] = h

    mod.set_axon_ntff_profile_hook = _set
    _sys.modules["antenv.axon_hooks"] = mod


def run(inputs, adj, op_emb, weight, attn_w, attn_b, self_op_emb, trace=False):
    if trace:
        _ensure_ntff_hook()
    from concourse.bass_utils import run_bass_kernel_spmd

    nc = _get_nc()
    in_maps = [
        marshal_core(inputs, adj, op_emb, weight, attn_w, attn_b, self_op_emb, c)
        for c in range(NCORES)
    ]
    res = run_bass_kernel_spmd(nc, in_maps, core_ids=list(range(NCORES)), trace=trace)
    out = np.concatenate(
        [res.results[c]["out"].reshape(BPC, N, OUT_F) for c in range(NCORES)], axis=0)
    return np.ascontiguousarray(out, np.float32), res


def kernel(inputs, adj, op_emb, weight, attn_w, attn_b, self_op_emb):
    out, _ = run(inputs, adj, op_emb, weight, attn_w, attn_b, self_op_emb, trace=False)
    return out
